# revision 20
# baseline (speedup 1.0000x reference)
"""Trainium2 Bass kernel for nn_AllPassMORRCirculantConv2d.

Math: out[n, (pp,t)] = sum_q scale_q * tr(phase[n,pp,q,t]) with
  tr(x) = (A^2 - 2*rho*cos x + R^2) / (1 - 2*rho*cos x + rho^2),  rho = A*R.
Since sum_q scale_q = 0 (differential rails), this reduces to
  out = -(C1/(2*rho)) * sum_q scale_q / (K - cos(phase_q)),
  C1 = (1-A^2)(1-R^2),  K = (1+rho^2)/(2*rho).

Per 512-pixel tile and q-pair j the device pipeline is:
  PE   : v = (phase/2pi) via block-circulant matmul (weights pre-scaled on host)
  ACT  : t = Identity(v + MAGIC)            -> MAGIC + round(v)  (fp32 RTNE)
  DVE  : y0 = (t * -1 + MAGIC) + v          -> v - round(v)      (affine_then_add)
  DVE  : u  = range-wrap(y0 - 1/4 into [-1/4, 1/4] by period 1)  (add_range_wrap)
  ACT  : c  = Sin(2pi*(1-eps) * u)          -> -cos(phase)
  DVE  : z  = c + K ; r = 1/z               (tensor_scalar_add + reciprocal_approx_fast)
  PE   : out_psum += S_j^T @ r              (q-reduction with host-built scale diag)

Data-parallel over the batch: core b handles image b. No collectives.
"""

import sys

for _p in ("/opt/trn_rl_repo", "/opt/pypackages"):
    if _p not in sys.path:
        sys.path.insert(0, _p)

import numpy as np

# --- problem constants (hardcoded; kernel.py must be self-contained) ---
A_ = 0.987
R_ = 0.99
RHO = A_ * R_
C1TR = (1.0 - A_ * A_) * (1.0 - R_ * R_)
C2TR = 1.0 + RHO * RHO
KCONST = C2TR / (2.0 * RHO)          # ~1.000268
BETA = -C1TR / (2.0 * RHO)           # folded into the q-reduction weights
CSC = 1.0 / (2.0 * np.pi)            # folded into the circulant weights
MAGIC = float(np.float32(1.5 * 2 ** 23))
SIN_SCALE = float(np.float32(2.0 * np.pi * (1.0 - 2.0 ** -22)))

B_, IN_C, H_, W_ = 8, 32, 64, 64
OUT_C, MB = 64, 8
Q_, P_ = 36, 8                        # grid dims; wic = 288
NPAIR = Q_ // 2                       # 18 q-pairs -> K=16 matmuls
L_ = H_ * W_                          # 4096 pixels per image
TPIX = 512                            # pixels per tile (fp32 matmul N max)
NTILE = L_ // TPIX


def host_prep(weight: np.ndarray, morr_output_scale: np.ndarray):
    """Build the two small replicated device matrices from the raw params.

    lhsT [16, 18*128]: block-circulant |w| * (1/2pi) for the phase matmul.
      lhsT[h*8+s, j*128 + h*64 + pp*8 + t] = CSC * |w|[pp, 2j+h, (t-s) % 8]
    smat [128, 18*64]: q-reduction diagonal-ish weights.
      smat[h*64+u, j*64 + u] = BETA * scalevec[2j+h]
    """
    wabs = np.abs(weight.astype(np.float64))                    # [8, 36, 8]
    s_i = np.arange(8)
    idx = (s_i[None, :] - s_i[:, None]) % 8                     # [s, t] -> (t-s)%8
    circ = wabs[:, :, idx]                                      # [pp, q, s, t]
    # K=32 blocks: pair j occupies rows (j%2)*16 .. +16 (rest zero) so the
    # matmul rhs can read the 32-aligned rep rows [32*(j//2), +32).
    lhsT = np.zeros((32, NPAIR, 128), np.float64)
    for j in range(NPAIR):
        ro = (j % 2) * 16
        for h in range(2):
            q = 2 * j + h
            for pp in range(P_):
                lhsT[
                    ro + h * 8: ro + h * 8 + 8,
                    j,
                    h * 64 + pp * 8: h * 64 + pp * 8 + 8,
                ] = CSC * circ[pp, q]
    lhsT = np.ascontiguousarray(lhsT.reshape(32, NPAIR * 128).astype(np.float32))

    sv = morr_output_scale.astype(np.float64)
    scalevec = np.concatenate([sv[:-1], -sv[:-1]])              # [36], q even branch
    smat = np.zeros((128, NPAIR, 64), np.float64)
    u_i = np.arange(64)
    for j in range(NPAIR):
        for h in range(2):
            smat[h * 64 + u_i, j, u_i] = BETA * scalevec[2 * j + h]
    smat = np.ascontiguousarray(smat.reshape(128, NPAIR * 64).astype(np.float32))
    return lhsT, smat


def host_unfold_sq(x_img: np.ndarray) -> np.ndarray:
    """numpy mirror of the on-device unfold+square: rep[w, l] (mini-model only)."""
    xsq = np.zeros((IN_C, 66, 66), np.float32)
    xsq[:, 1:65, 1:65] = (x_img * x_img).astype(np.float32)
    rep = np.zeros((288, L_), np.float32)
    for c in range(IN_C):
        for kh in range(3):
            for kw in range(3):
                w = c * 9 + kh * 3 + kw
                rep[w] = xsq[c, kh:kh + 64, kw:kw + 64].reshape(-1)
    return rep


def minimodel(x_img, lhsT, smat):
    """Pure-numpy fp32 mirror of the device pipeline for one image (debugging)."""
    rep = host_unfold_sq(x_img)
    out = np.zeros((64, L_), np.float32)
    for j in range(NPAIR):
        lh = lhsT[:, j * 128:(j + 1) * 128]                     # [32, 128]
        r0 = 32 * (j // 2)
        rhs = rep[r0:r0 + 32, :]                                # [32, L]
        v = (lh.T.astype(np.float32) @ rhs).astype(np.float32)  # [128, L]
        t = np.float32(v + np.float32(MAGIC))
        y0 = np.float32(np.float32(-t + np.float32(MAGIC)) + v)
        yy = np.float32(y0 - np.float32(0.25))
        u = np.float32(yy + np.float32(1.0) * ((yy < -0.5).astype(np.float32)
                                               - (yy > 0.5).astype(np.float32)))
        assert np.abs(u * np.float32(SIN_SCALE)).max() <= np.pi, "sin range"
        c = np.sin(np.float32(u * np.float32(SIN_SCALE)), dtype=np.float32)
        z = np.float32(c + np.float32(KCONST))
        r = np.float32(1.0 / z)                                 # device: ~51 ULP approx
        sm = smat[:, j * 64:(j + 1) * 64]                       # [128, 64]
        out += (sm.T @ r).astype(np.float32)
    return out


# ----------------------------------------------------------------------------
# device kernel build
# ----------------------------------------------------------------------------

def _build(tc, dram):
    from contextlib import ExitStack
    import concourse.mybir as mybir

    ctx = ExitStack()
    nc = tc.nc
    F32 = mybir.dt.float32
    AF = mybir.ActivationFunctionType

    cpool = ctx.enter_context(tc.tile_pool(name="const", bufs=1))
    x_sb = cpool.tile([IN_C, L_], F32)
    nc.sync.dma_start(x_sb[:], dram["x"][:])
    lh_sb = cpool.tile([32, NPAIR * 128], F32)
    nc.sync.dma_start(lh_sb[:], dram["lhsT"][:])
    sm_sb = cpool.tile([128, NPAIR * 64], F32)
    nc.sync.dma_start(sm_sb[:], dram["smat"][:])

    # per-partition constant columns for activation bias/scale operands
    cst_magic = cpool.tile([128, 1], F32)
    nc.gpsimd.memset(cst_magic[:], MAGIC)
    cst_sinsc = cpool.tile([128, 1], F32)
    nc.gpsimd.memset(cst_sinsc[:], SIN_SCALE)

    # squared, zero-padded input plane: [32, 66*66]
    xsq = cpool.tile([IN_C, 66 * 66], F32)
    nc.vector.memset(xsq[:], 0.0)
    x3 = x_sb[:].rearrange("p (a b) -> p a b", b=W_)
    xq3 = xsq[:].rearrange("p (a b) -> p a b", b=66)
    nc.scalar.activation(xq3[:, 1:65, 1:65], x3, AF.Square)

    # unfold replication via DRAM: rep row w = 9c + kappa holds the
    # (kh,kw)-shifted 64x64 window of channel c. DRAM allows the stepped row
    # writes; per-pixel-tile 32-row blocks are DMA'd back contiguously.
    repd = dram["repd"]
    for kk in range(9):
        kh, kw = kk // 3, kk % 3
        dst = repd[kk::9].rearrange("p (a b) -> p a b", b=W_)   # rows 9c+kk
        src = xq3[:, kh:kh + 64, kw:kw + 64]
        nc.sync.dma_start(dst, src)

    vpool = ctx.enter_context(tc.tile_pool(name="vps", bufs=2, space="PSUM"))
    opool = ctx.enter_context(tc.tile_pool(name="ops", bufs=2, space="PSUM"))
    wpool = ctx.enter_context(tc.tile_pool(name="work", bufs=2))
    outp = ctx.enter_context(tc.tile_pool(name="outsb", bufs=2))

    rpool = ctx.enter_context(tc.tile_pool(name="repsb", bufs=6))

    for T in range(NTILE):
        px = bass_ts(T, TPIX)
        o_ps = opool.tile([64, TPIX], F32)
        for g in range(NPAIR // 2):
            rep_sb = rpool.tile([32, TPIX], F32)
            nc.sync.dma_start(rep_sb[:], repd[32 * g:32 * g + 32, px])
            v_ps = vpool.tile([128, 2 * TPIX], F32)
            for e in range(2):
                j = 2 * g + e
                nc.tensor.matmul(
                    v_ps[:, e * TPIX:(e + 1) * TPIX],
                    lh_sb[:, j * 128:(j + 1) * 128],
                    rep_sb[:],
                    start=True,
                    stop=True,
                )
            t_sb = wpool.tile([128, 2 * TPIX], F32)
            nc.scalar.activation(t_sb[:], v_ps[:], AF.Identity, bias=cst_magic[:])
            y_sb = wpool.tile([128, 2 * TPIX], F32)
            nc.vector.affine_then_add(
                y_sb[:], t_sb[:], v_ps[:], scale=-1.0, bias=MAGIC
            )
            u_sb = wpool.tile([128, 2 * TPIX], F32)
            nc.vector.add_range_wrap(
                u_sb[:], y_sb[:], shift=-0.25, bound=0.5, period=1.0
            )
            c_sb = wpool.tile([128, 2 * TPIX], F32)
            nc.scalar.activation(c_sb[:], u_sb[:], AF.Sin, scale=cst_sinsc[:])
            z_sb = wpool.tile([128, 2 * TPIX], F32)
            nc.vector.tensor_scalar_add(z_sb[:], c_sb[:], float(KCONST))
            r_sb = wpool.tile([128, 2 * TPIX], F32)
            nc.vector.reciprocal_approx_fast(r_sb[:], z_sb[:])
            for e in range(2):
                j = 2 * g + e
                nc.tensor.matmul(
                    o_ps[:],
                    sm_sb[:, j * 64:(j + 1) * 64],
                    r_sb[:, e * TPIX:(e + 1) * TPIX],
                    start=(j == 0),
                    stop=(j == NPAIR - 1),
                )
        o_sb = outp.tile([64, TPIX], F32)
        nc.scalar.copy(o_sb[:], o_ps[:])
        nc.sync.dma_start(dram["out"][:, px], o_sb[:])
    ctx.close()


def bass_ts(i, size):
    return slice(i * size, (i + 1) * size)


_COMPILED = {}


def _get_graph():
    if "nc" in _COMPILED:
        return _COMPILED["nc"]
    import concourse.bacc as bacc
    import concourse.tile as tile
    import concourse.mybir as mybir

    F32 = mybir.dt.float32
    nc = bacc.Bacc(
        "TRN2", target_bir_lowering=False, debug=False, num_devices=8
    )
    dram = {
        "x": nc.dram_tensor("x", [IN_C, L_], F32, kind="ExternalInput").ap(),
        "lhsT": nc.dram_tensor(
            "lhsT", [32, NPAIR * 128], F32, kind="ExternalInput"
        ).ap(),
        "smat": nc.dram_tensor(
            "smat", [128, NPAIR * 64], F32, kind="ExternalInput"
        ).ap(),
        "out": nc.dram_tensor(
            "out", [OUT_C, L_], F32, kind="ExternalOutput"
        ).ap(),
        "repd": nc.dram_tensor("repd", [288, L_], F32, kind="Internal").ap(),
    }
    with tile.TileContext(nc) as tc:
        _build(tc, dram)
    nc.compile()
    _COMPILED["nc"] = nc
    return nc


def _run(x, weight, morr_output_scale, trace=False):
    from concourse import bass_utils

    lhsT, smat = host_prep(weight, morr_output_scale)
    in_maps = [
        {
            "x": np.ascontiguousarray(x[b].reshape(IN_C, L_).astype(np.float32)),
            "lhsT": lhsT,
            "smat": smat,
        }
        for b in range(B_)
    ]
    nc = _get_graph()
    res = bass_utils.run_bass_kernel_spmd(
        nc, in_maps, core_ids=list(range(8)), trace=trace
    )
    out = np.stack([res.results[b]["out"].reshape(OUT_C, H_, W_) for b in range(B_)])
    return out.astype(np.float32), res


def kernel(x, weight, morr_output_scale):
    out, _ = _run(
        np.asarray(x), np.asarray(weight), np.asarray(morr_output_scale)
    )
    return out
